# revision 25
# baseline (speedup 1.0000x reference)
"""Trainium2 Bass kernel for nn_Attention_56899726738131.

Full attention with both outputs: (out [B,H,L,D], p_attn [B,H,L,L]).
  scores = Q K^T / sqrt(D); masked -> -1e9; softmax; p_attn zeroed at mask==0;
  out = p_attn @ V.

Sharding: B*H = 64 (b,h) pairs split across 8 cores (8 pairs each, batch-major
so each core needs exactly one batch's mask). Each core runs the same
single-core program on its slice (pure SPMD, no collectives).

Per-pair dataflow on one core (all matmuls bf16, softmax numerics in fp32):
  - Q^T/K^T built by PE transposes of cast-to-bf16 loads.
  - S   = Q K^T   (straight [q,k] layout, PSUM fp32)  -> exp on ScalarE (bf16 out)
  - mask multiply + row-sum fused on VectorE (tensor_tensor_reduce)
  - p_attn tile = E_masked * (1/rowsum)  (per-partition scalar, fp32 out)
  - S^T = K Q^T   ([k,q] layout) -> exp -> mask multiply (VectorE/GpSimd)
  - O^T = sum_k V_chunk.T @ W^T_chunk (PSUM accumulate), transpose back on PE,
    scale by 1/rowsum.
Masking is multiplicative (exact zeros, matching the reference's
where(mask==0, 0, softmax)); no max-subtraction is needed because
scores ~ N(0,1) (exp never overflows) and the reference's -1e9 entries
underflow to exactly 0 in its own softmax, so denominators match.
"""

import numpy as np

import concourse.bass as bass
import concourse.bacc as bacc
import concourse.mybir as mybir
import concourse.tile as tile
from concourse.bass_utils import run_bass_kernel_spmd
from concourse.masks import make_identity

B, H, L, D = 4, 16, 1024, 64
N_CORES = 8
PAIRS = (B * H) // N_CORES  # 8 (b,h) pairs per core
P = 128                     # partition tile
NC_ = L // P                # 8 chunks of 128 along either L axis
SCALE = 1.0 / np.sqrt(np.float32(D))  # 0.125

F32 = mybir.dt.float32
BF16 = mybir.dt.bfloat16
I32 = mybir.dt.int32
AF = mybir.ActivationFunctionType
ALU = mybir.AluOpType


def build_nc(n_pairs=PAIRS):
    nc = bacc.Bacc("TRN2")

    q_d = nc.dram_tensor("q", [n_pairs, L, D], F32, kind="ExternalInput")
    k_d = nc.dram_tensor("k", [n_pairs, L, D], F32, kind="ExternalInput")
    v_d = nc.dram_tensor("v", [n_pairs, L, D], F32, kind="ExternalInput")
    m_d = nc.dram_tensor("mask", [L, L], I32, kind="ExternalInput")
    # One output tensor per pair: avoids DRAM WAW tracking between pairs'
    # stores (walrus allows at most one sync wait per DMA instruction).
    o_ds = [nc.dram_tensor(f"o{i}", [L, D], F32, kind="ExternalOutput")
            for i in range(n_pairs)]
    p_ds = [nc.dram_tensor(f"p{i}", [L, L], F32, kind="ExternalOutput")
            for i in range(n_pairs)]

    with tile.TileContext(nc) as tc:
        with (
            tc.tile_pool(name="consts", bufs=1) as consts,
            tc.tile_pool(name="masks", bufs=1) as masks,
            tc.tile_pool(name="qkv", bufs=3) as qkv,
            tc.tile_pool(name="qt", bufs=2) as qtp,
            tc.tile_pool(name="e", bufs=2 * NC_ + 2) as ep,
            tc.tile_pool(name="et", bufs=3) as etp,
            tc.tile_pool(name="pf", bufs=2) as pfp,
            tc.tile_pool(name="ot", bufs=2) as otp,
            tc.tile_pool(name="osb", bufs=2) as osb,
            tc.tile_pool(name="rr", bufs=4) as rrp,
            tc.tile_pool(name="psS", bufs=1, space="PSUM") as psS,
            tc.tile_pool(name="psST", bufs=1, space="PSUM") as psST,
            tc.tile_pool(name="psOT", bufs=1, space="PSUM") as psOT,
            tc.tile_pool(name="psT", bufs=1, space="PSUM") as psT,
        ):
            ident = consts.tile([P, P], BF16)
            make_identity(nc, ident)

            # ---- mask setup (once per core) ----
            # m_bf[:, qc, :] = bf16(mask[qc*P:(qc+1)*P, :])   straight [q,k]
            # mt_bf[:, kc, :] = transpose                     [k,q]
            m_bf = masks.tile([P, NC_, L], BF16)
            mt_bf = masks.tile([P, NC_, L], BF16)
            nc.gpsimd.dma_start(  # int32 -> bf16 cast during DMA (SWDGE)
                out=m_bf, in_=m_d.rearrange("(t p) k -> p t k", p=P))
            for qc in range(NC_):
                for kc in range(NC_):
                    pm = psT.tile([P, P], BF16, tag="ptr")
                    nc.tensor.transpose(
                        out=pm, in_=m_bf[:, qc, kc * P:(kc + 1) * P],
                        identity=ident)
                    nc.scalar.copy(
                        out=mt_bf[:, kc, qc * P:(qc + 1) * P], in_=pm)

            for idx in range(n_pairs):
                # ---- load + transpose Q, K; load V ----
                qbf = qkv.tile([P, NC_, D], BF16, tag="qbf")
                kbf = qkv.tile([P, NC_, D], BF16, tag="kbf")
                vbf = qkv.tile([P, NC_, D], BF16, tag="vbf")
                nc.gpsimd.dma_start(
                    out=qbf, in_=q_d[idx].rearrange("(t p) d -> p t d", p=P))
                nc.gpsimd.dma_start(
                    out=kbf, in_=k_d[idx].rearrange("(t p) d -> p t d", p=P))
                nc.gpsimd.dma_start(
                    out=vbf, in_=v_d[idx].rearrange("(t p) d -> p t d", p=P))

                qt = qtp.tile([D, L], BF16, tag="qt")
                kt = qtp.tile([D, L], BF16, tag="kt")
                for t in range(NC_):
                    pq = psT.tile([D, P], BF16, tag="ptr")
                    nc.tensor.transpose(out=pq, in_=qbf[:, t, :], identity=ident)
                    nc.vector.tensor_copy(out=qt[:, t * P:(t + 1) * P], in_=pq)
                    pk = psT.tile([D, P], BF16, tag="ptr")
                    nc.tensor.transpose(out=pk, in_=kbf[:, t, :], identity=ident)
                    nc.vector.tensor_copy(out=kt[:, t * P:(t + 1) * P], in_=pk)

                # ---- straight side: S = Q K^T, E = exp(S/8), r = rowsum(E*M) ----
                rr = rrp.tile([P, NC_], F32, tag="rr")
                rcp = rrp.tile([P, NC_], F32, tag="rcp")
                e_tiles = []
                for qc in range(NC_):
                    ps = psS.tile([P, L], F32)
                    for h in range(2):
                        hs = slice(h * (L // 2), (h + 1) * (L // 2))
                        nc.tensor.matmul(
                            out=ps[:, hs], lhsT=qt[:, qc * P:(qc + 1) * P],
                            rhs=kt[:, hs], start=True, stop=True)
                    e = ep.tile([P, L], BF16)
                    nc.scalar.activation(out=e, in_=ps, func=AF.Exp, scale=float(SCALE))
                    # (tensor_tensor_reduce wedges TRN2 hw; InstTensorScalarPtr
                    # with accum_out is the working equivalent)
                    nc.vector.scalar_tensor_tensor(
                        out=e, in0=e, scalar=1.0, in1=m_bf[:, qc, :],
                        op0=ALU.mult, op1=ALU.mult,
                        accum_out=rr[:, qc:qc + 1])
                    e_tiles.append(e)
                nc.vector.reciprocal(out=rcp, in_=rr)

                # ---- p_attn = E_masked / rowsum ----
                p_pair = pfp.tile([P, NC_, L], F32)
                for qc in range(NC_):
                    nc.vector.tensor_scalar_mul(
                        p_pair[:, qc, :], e_tiles[qc], rcp[:, qc:qc + 1])
                nc.sync.dma_start(
                    out=p_ds[idx].rearrange("(t p) k -> p t k", p=P), in_=p_pair)

                # ---- T side: S^T = K Q^T, W^T = exp(S^T/8) * M^T; O^T = V^T W^T ----
                otps = psOT.tile([D, L], F32)
                for kc in range(NC_):
                    pst = psST.tile([P, L], F32)
                    for h in range(2):
                        hs = slice(h * (L // 2), (h + 1) * (L // 2))
                        nc.tensor.matmul(
                            out=pst[:, hs], lhsT=kt[:, kc * P:(kc + 1) * P],
                            rhs=qt[:, hs], start=True, stop=True)
                    et = etp.tile([P, L], BF16)
                    nc.scalar.activation(out=et, in_=pst, func=AF.Exp, scale=float(SCALE))
                    eng = nc.gpsimd if kc % 2 == 0 else nc.vector
                    eng.tensor_tensor(out=et, in0=et, in1=mt_bf[:, kc, :], op=ALU.mult)
                    for h in range(2):
                        hs = slice(h * (L // 2), (h + 1) * (L // 2))
                        nc.tensor.matmul(
                            out=otps[:, hs], lhsT=vbf[:, kc, :], rhs=et[:, hs],
                            start=(kc == 0), stop=(kc == NC_ - 1))

                # ---- O = (O^T)^T / rowsum ----
                otb = otp.tile([D, L], BF16)
                nc.vector.tensor_copy(out=otb, in_=otps)
                o_sb = osb.tile([P, NC_, D], F32)
                for t in range(NC_):
                    po = psT.tile([P, D], BF16, tag="pot")
                    nc.tensor.transpose(
                        out=po, in_=otb[:, t * P:(t + 1) * P],
                        identity=ident[:D, :D])
                    nc.vector.tensor_scalar_mul(o_sb[:, t, :], po, rcp[:, t:t + 1])
                nc.sync.dma_start(
                    out=o_ds[idx].rearrange("(t p) d -> p t d", p=P), in_=o_sb)

    if not nc.is_finalized():
        nc.finalize()  # Bacc: wait splitting, register allocation, DCE
    return nc


_nc_cache = {}


def _get_nc(n_pairs=PAIRS):
    if n_pairs not in _nc_cache:
        _nc_cache[n_pairs] = build_nc(n_pairs)
    return _nc_cache[n_pairs]


def _shard(query, key, value, mask):
    """Full inputs -> per-core in_maps. Core c: batch c//2, heads (c%2)*8..+8."""
    query = np.asarray(query, dtype=np.float32)
    key = np.asarray(key, dtype=np.float32)
    value = np.asarray(value, dtype=np.float32)
    mask = np.asarray(mask, dtype=np.int32)
    in_maps = []
    for c in range(N_CORES):
        b, h0 = c // 2, (c % 2) * PAIRS
        in_maps.append({
            "q": np.ascontiguousarray(query[b, h0:h0 + PAIRS]),
            "k": np.ascontiguousarray(key[b, h0:h0 + PAIRS]),
            "v": np.ascontiguousarray(value[b, h0:h0 + PAIRS]),
            "mask": np.ascontiguousarray(mask[b, 0]),
        })
    return in_maps


def _gather(results):
    out = np.empty((B, H, L, D), np.float32)
    p = np.empty((B, H, L, L), np.float32)
    for c, r in enumerate(results):
        b, h0 = c // 2, (c % 2) * PAIRS
        for i in range(PAIRS):
            out[b, h0 + i] = r[f"o{i}"]
            p[b, h0 + i] = r[f"p{i}"]
    return out, p


def run(query, key, value, mask, trace=False, **kw):
    nc = _get_nc()
    res = run_bass_kernel_spmd(
        nc, _shard(query, key, value, mask),
        core_ids=list(range(N_CORES)), trace=trace, **kw)
    return _gather(res.results), res


def kernel(query, key, value, mask):
    (out, p), _ = run(query, key, value, mask)
    return out, p


# revision 30
# speedup vs baseline: 385.1525x; 385.1525x over previous
"""Trainium2 Bass kernel for nn_Attention_56899726738131.

Full attention with both outputs: (out [B,H,L,D], p_attn [B,H,L,L]).
  scores = Q K^T / sqrt(D); masked -> -1e9; softmax; p_attn zeroed at mask==0;
  out = p_attn @ V.

Sharding: B*H = 64 (b,h) pairs split across 8 cores (8 pairs each, batch-major
so each core needs exactly one batch's mask). Each core runs the same
single-core program on its slice (pure SPMD, no collectives).

Per-pair dataflow on one core (all matmuls bf16, softmax numerics in fp32):
  - Q^T/K^T built by PE transposes of cast-to-bf16 loads.
  - S   = Q K^T   (straight [q,k] layout, PSUM fp32)  -> exp on ScalarE (bf16 out)
  - mask multiply + row-sum fused on VectorE (tensor_tensor_reduce)
  - p_attn tile = E_masked * (1/rowsum)  (per-partition scalar, fp32 out)
  - S^T = K Q^T   ([k,q] layout) -> exp -> mask multiply (VectorE/GpSimd)
  - O^T = sum_k V_chunk.T @ W^T_chunk (PSUM accumulate), transpose back on PE,
    scale by 1/rowsum.
Masking is multiplicative (exact zeros, matching the reference's
where(mask==0, 0, softmax)); no max-subtraction is needed because
scores ~ N(0,1) (exp never overflows) and the reference's -1e9 entries
underflow to exactly 0 in its own softmax, so denominators match.
"""

import numpy as np

import concourse.bass as bass
import concourse.bacc as bacc
import concourse.mybir as mybir
import concourse.tile as tile
from concourse.bass_utils import run_bass_kernel_spmd
from concourse.masks import make_identity

B, H, L, D = 4, 16, 1024, 64
N_CORES = 8
PAIRS = (B * H) // N_CORES  # 8 (b,h) pairs per core
P = 128                     # partition tile
NC_ = L // P                # 8 chunks of 128 along either L axis
SCALE = 1.0 / np.sqrt(np.float32(D))  # 0.125

F32 = mybir.dt.float32
BF16 = mybir.dt.bfloat16
I32 = mybir.dt.int32
AF = mybir.ActivationFunctionType
ALU = mybir.AluOpType


FEAT = "full"  # io | qkt | straight | full  (perf bisection)


def build_nc(n_pairs=PAIRS, feat=None, reps=1):
    feat = feat or FEAT
    lvl = ["io", "qkt", "straight", "full"].index(feat)
    nc = bacc.Bacc("TRN2")

    q_d = nc.dram_tensor("q", [n_pairs, L, D], F32, kind="ExternalInput")
    k_d = nc.dram_tensor("k", [n_pairs, L, D], F32, kind="ExternalInput")
    v_d = nc.dram_tensor("v", [n_pairs, L, D], F32, kind="ExternalInput")
    m_d = nc.dram_tensor("mask", [L, L], I32, kind="ExternalInput")
    # One output tensor per pair: avoids DRAM WAW tracking between pairs'
    # stores (walrus allows at most one sync wait per DMA instruction).
    o_ds = [nc.dram_tensor(f"o{i}", [L, D], F32, kind="ExternalOutput")
            for i in range(n_pairs)]
    p_ds = [nc.dram_tensor(f"p{i}", [L, L], F32, kind="ExternalOutput")
            for i in range(n_pairs)]

    with tile.TileContext(nc) as tc:
        with (
            tc.tile_pool(name="consts", bufs=1) as consts,
            tc.tile_pool(name="masks", bufs=1) as masks,
            tc.tile_pool(name="qkv", bufs=3) as qkv,
            tc.tile_pool(name="qt", bufs=2) as qtp,
            tc.tile_pool(name="e", bufs=2 * NC_ + 2) as ep,
            tc.tile_pool(name="et", bufs=3) as etp,
            tc.tile_pool(name="pf", bufs=2) as pfp,
            tc.tile_pool(name="ot", bufs=2) as otp,
            tc.tile_pool(name="osb", bufs=2) as osb,
            tc.tile_pool(name="rr", bufs=4) as rrp,
            tc.tile_pool(name="psS", bufs=2, space="PSUM") as psS,
            tc.tile_pool(name="psOT", bufs=1, space="PSUM") as psOT,
            tc.tile_pool(name="psT", bufs=2, space="PSUM") as psT,
        ):
            ident = consts.tile([P, P], BF16)
            make_identity(nc, ident)

            for _rep in range(reps):
                _emit_program(nc, tc, lvl, n_pairs, ident,
                              masks, qkv, qtp, ep, etp, pfp, otp, osb, rrp,
                              psS, psS, psOT, psT,
                              q_d, k_d, v_d, m_d, o_ds, p_ds)

    if not nc.is_finalized():
        nc.finalize()  # Bacc: wait splitting, register allocation, DCE
    return nc


def _emit_program(nc, tc, lvl, n_pairs, ident,
                  masks, qkv, qtp, ep, etp, pfp, otp, osb, rrp,
                  psS, psST, psOT, psT,
                  q_d, k_d, v_d, m_d, o_ds, p_ds):
    if True:
        if True:
            # ---- mask setup (once per core) ----
            # m_bf[:, qc, :] = bf16(mask[qc*P:(qc+1)*P, :])   straight [q,k]
            # mt_bf[:, kc, :] = transpose                     [k,q]
            m_bf = masks.tile([P, NC_, L], BF16)
            mt_bf = masks.tile([P, NC_, L], BF16)
            nc.gpsimd.dma_start(  # int32 -> bf16 cast during DMA (SWDGE)
                out=m_bf, in_=m_d.rearrange("(t p) k -> p t k", p=P))
            for qc in range(NC_ if lvl >= 3 else 0):
                for kc in range(NC_):
                    pm = psT.tile([P, P], BF16, tag="tr")
                    nc.tensor.transpose(
                        out=pm, in_=m_bf[:, qc, kc * P:(kc + 1) * P],
                        identity=ident)
                    nc.scalar.copy(
                        out=mt_bf[:, kc, qc * P:(qc + 1) * P], in_=pm)

            for idx in range(n_pairs):
                # ---- load + transpose Q, K; load V ----
                qbf = qkv.tile([P, NC_, D], BF16, tag="qbf")
                kbf = qkv.tile([P, NC_, D], BF16, tag="kbf")
                vbf = qkv.tile([P, NC_, D], BF16, tag="vbf")
                nc.gpsimd.dma_start(
                    out=qbf, in_=q_d[idx].rearrange("(t p) d -> p t d", p=P))
                nc.gpsimd.dma_start(
                    out=kbf, in_=k_d[idx].rearrange("(t p) d -> p t d", p=P))
                nc.gpsimd.dma_start(
                    out=vbf, in_=v_d[idx].rearrange("(t p) d -> p t d", p=P))

                qt = qtp.tile([D, L], BF16, tag="qt")
                kt = qtp.tile([D, L], BF16, tag="kt")
                for t in range(NC_ if lvl >= 1 else 0):
                    pq = psT.tile([P, P], BF16, tag="tr")
                    nc.tensor.transpose(out=pq[:D, :], in_=qbf[:, t, :], identity=ident)
                    nc.vector.tensor_copy(out=qt[:, t * P:(t + 1) * P], in_=pq[:D, :])
                    pk = psT.tile([P, P], BF16, tag="tr")
                    nc.tensor.transpose(out=pk[:D, :], in_=kbf[:, t, :], identity=ident)
                    nc.vector.tensor_copy(out=kt[:, t * P:(t + 1) * P], in_=pk[:D, :])

                # ---- straight side: S = Q K^T, E = exp(S/8), r = rowsum(E*M) ----
                rr = rrp.tile([P, NC_], F32, tag="rr")
                rcp = rrp.tile([P, NC_], F32, tag="rcp")
                e_tiles = []
                for qc in range(NC_ if lvl >= 2 else 0):
                    ps = psS.tile([P, L], F32, tag='s')
                    for h in range(2):
                        hs = slice(h * (L // 2), (h + 1) * (L // 2))
                        nc.tensor.matmul(
                            out=ps[:, hs], lhsT=qt[:, qc * P:(qc + 1) * P],
                            rhs=kt[:, hs], start=True, stop=True)
                    e = ep.tile([P, L], BF16)
                    nc.scalar.activation(out=e, in_=ps, func=AF.Exp, scale=float(SCALE))
                    # (tensor_tensor_reduce wedges TRN2 hw; InstTensorScalarPtr
                    # with accum_out is the working equivalent)
                    nc.vector.scalar_tensor_tensor(
                        out=e, in0=e, scalar=1.0, in1=m_bf[:, qc, :],
                        op0=ALU.mult, op1=ALU.mult,
                        accum_out=rr[:, qc:qc + 1])
                    e_tiles.append(e)
                if lvl >= 2:
                    nc.vector.reciprocal(out=rcp, in_=rr)
                else:
                    nc.vector.memset(rcp, 0.0)

                # ---- p_attn = E_masked / rowsum ----
                p_pair = pfp.tile([P, NC_, L], F32)
                for qc in range(NC_):
                    if lvl >= 2:
                        nc.vector.tensor_scalar_mul(
                            p_pair[:, qc, :], e_tiles[qc], rcp[:, qc:qc + 1])
                    else:
                        nc.vector.memset(p_pair[:, qc, :], 0.0)
                nc.sync.dma_start(
                    out=p_ds[idx].rearrange("(t p) k -> p t k", p=P), in_=p_pair)

                # ---- T side: S^T = K Q^T, W^T = exp(S^T/8) * M^T; O^T = V^T W^T ----
                otps = psOT.tile([D, L], F32)
                if lvl < 3:
                    nc.vector.memset(otps, 0.0)
                for kc in range(NC_ if lvl >= 3 else 0):
                    pst = psST.tile([P, L], F32, tag='s')
                    for h in range(2):
                        hs = slice(h * (L // 2), (h + 1) * (L // 2))
                        nc.tensor.matmul(
                            out=pst[:, hs], lhsT=kt[:, kc * P:(kc + 1) * P],
                            rhs=qt[:, hs], start=True, stop=True)
                    et = etp.tile([P, L], BF16)
                    nc.scalar.activation(out=et, in_=pst, func=AF.Exp, scale=float(SCALE))
                    eng = nc.gpsimd if kc % 2 == 0 else nc.vector
                    eng.tensor_tensor(out=et, in0=et, in1=mt_bf[:, kc, :], op=ALU.mult)
                    for h in range(2):
                        hs = slice(h * (L // 2), (h + 1) * (L // 2))
                        nc.tensor.matmul(
                            out=otps[:, hs], lhsT=vbf[:, kc, :], rhs=et[:, hs],
                            start=(kc == 0), stop=(kc == NC_ - 1))

                # ---- O = (O^T)^T / rowsum ----
                otb = otp.tile([D, L], BF16)
                nc.vector.tensor_copy(out=otb, in_=otps)
                o_sb = osb.tile([P, NC_, D], F32)
                for t in range(NC_):
                    po = psT.tile([P, P], BF16, tag="tr")
                    nc.tensor.transpose(
                        out=po[:, :D], in_=otb[:, t * P:(t + 1) * P],
                        identity=ident[:D, :D])
                    nc.vector.tensor_scalar_mul(o_sb[:, t, :], po[:, :D], rcp[:, t:t + 1])
                nc.sync.dma_start(
                    out=o_ds[idx].rearrange("(t p) d -> p t d", p=P), in_=o_sb)


_nc_cache = {}


def _get_nc(n_pairs=PAIRS):
    if n_pairs not in _nc_cache:
        _nc_cache[n_pairs] = build_nc(n_pairs)
    return _nc_cache[n_pairs]


def _shard(query, key, value, mask):
    """Full inputs -> per-core in_maps. Core c: batch c//2, heads (c%2)*8..+8."""
    query = np.asarray(query, dtype=np.float32)
    key = np.asarray(key, dtype=np.float32)
    value = np.asarray(value, dtype=np.float32)
    mask = np.asarray(mask, dtype=np.int32)
    in_maps = []
    for c in range(N_CORES):
        b, h0 = c // 2, (c % 2) * PAIRS
        in_maps.append({
            "q": np.ascontiguousarray(query[b, h0:h0 + PAIRS]),
            "k": np.ascontiguousarray(key[b, h0:h0 + PAIRS]),
            "v": np.ascontiguousarray(value[b, h0:h0 + PAIRS]),
            "mask": np.ascontiguousarray(mask[b, 0]),
        })
    return in_maps


def _gather(results):
    out = np.empty((B, H, L, D), np.float32)
    p = np.empty((B, H, L, L), np.float32)
    for c, r in enumerate(results):
        b, h0 = c // 2, (c % 2) * PAIRS
        for i in range(PAIRS):
            out[b, h0 + i] = r[f"o{i}"]
            p[b, h0 + i] = r[f"p{i}"]
    return out, p


def run(query, key, value, mask, trace=False, **kw):
    nc = _get_nc()
    res = run_bass_kernel_spmd(
        nc, _shard(query, key, value, mask),
        core_ids=list(range(N_CORES)), trace=trace, **kw)
    return _gather(res.results), res


def kernel(query, key, value, mask):
    (out, p), _ = run(query, key, value, mask)
    return out, p
